# revision 43
# baseline (speedup 1.0000x reference)
"""Trainium2 Bass kernel for BoundaryLoss (data-parallel over batch).

Math (per batch sample b):
  mask  = boundary mask of target: 1 iff the cv2-clipped 5x5 window is
          non-uniform (equals the reference's per-class dilate/erode union).
  ce    = logsumexp_c(pred) - pred[t]
  wsum  = sum(mask * ce);  msum = sum(mask)
  per_sample = msum > 0 ? wsum/max(msum,1) : wsum/(H*W);  out = mean_b

Device algorithm (one sample per core), everything in "layout B"
[128, (r=4 rows, 512)] where partition p holds rows 4p..4p+3 (8KB
contiguous DMA runs, ~420 GB/s measured):
  - S = sum_c exp(pred_c): exp on ACT (fp16 out), summed over classes by
    identity-matmul PSUM accumulation on PE (4 banks).
  - G = exp(pred)[t] via per-class o = (t==c)*e: eq on DVE (rows 0-1,
    tensor_scalar 4x) + GpSimd (rows 2-3, otherwise idle), mult on DVE
    (tensor_tensor fp16 2x), PSUM-accumulated on PE (4 banks).
    ln G = pred[t] recovers the gather.
  - mask via edge indicators, entirely in layout B (no transposes):
      eh = horiz class-change, ev = vert class-change (in-partition rows
      + one partition-shift SBUF->SBUF DMA for the 4p+3 -> 4(p+1) seam);
      window-or = separable max pools: horizontal pools as shifted
      tensor_tensor max ops in 520-wide zero-padded buffers (flat APs,
      borders absorb row crossings); vertical pools as FD-512 row maxes
      with the p+-1 cross-partition terms fetched by small partition-
      shift SBUF->SBUF DMAs on the gpsimd queue.
    mask = max(A5, B4); msum accumulated on the fly.
  - finals: sum(mask*ln S) and sum(mask*ln G) via half-split Ln (ACT,
    fp16) + scalar_tensor_tensor accumulation (DVE), partition-reduce
    via ones-matmuls reusing the S PSUM bank; DMA out [1,32].
Host combines the per-core outputs.
"""

import numpy as np

B = 8
C = 21
H = 512
W = 512
N_CORES = 8
CHUNK = 2  # pred planes per DMA
G4 = 4  # row groups (H = 128 * G4)
PW = 520  # padded width of horizontal pooling buffers; data cols [2, 514)
POOL_EQ_ROWS = 0  # rows of the per-class eq on GpSimd (measured: Q7
# tensor_scalar is ~27x slower than DVE and contends for DVE's SBUF port)

_CACHE = {}


def _build_nc():
    from contextlib import ExitStack

    import concourse.bacc as bacc
    import concourse.tile as tile
    from concourse import mybir
    from concourse.masks import make_identity

    dt = mybir.dt
    Alu = mybir.AluOpType
    Act = mybir.ActivationFunctionType

    nc = bacc.Bacc("TRN2", target_bir_lowering=False, debug=False,
                   num_devices=N_CORES)

    pred = nc.dram_tensor("pred", [C, H, W], dt.float32, kind="ExternalInput")
    target = nc.dram_tensor("target", [H, W], dt.int32, kind="ExternalInput")
    out = nc.dram_tensor("out", [1, 32], dt.float32, kind="ExternalOutput")

    with tile.TileContext(nc) as tc, ExitStack() as ctx:
        consts = ctx.enter_context(tc.tile_pool(name="consts", bufs=1))
        keep = ctx.enter_context(tc.tile_pool(name="keep", bufs=1))
        mp = ctx.enter_context(tc.tile_pool(name="maskpool", bufs=1))
        ms = ctx.enter_context(tc.tile_pool(name="maskscratch", bufs=1))
        ppool = ctx.enter_context(tc.tile_pool(name="pp", bufs=3))
        epool = ctx.enter_context(tc.tile_pool(name="ep", bufs=4))
        qpool = ctx.enter_context(tc.tile_pool(name="qp", bufs=4))
        jpool = ctx.enter_context(tc.tile_pool(name="jp", bufs=2))
        opool = ctx.enter_context(tc.tile_pool(name="op", bufs=4))
        fin = ctx.enter_context(tc.tile_pool(name="fin", bufs=1))
        sgp = ctx.enter_context(tc.tile_pool(name="sgpsum", bufs=1,
                                             space="PSUM"))

        ident = consts.tile([128, 128], dt.float16)
        make_identity(nc, ident)
        ones = consts.tile([128, 1], dt.float32)
        nc.gpsimd.memset(ones, 1.0)
        warm = consts.tile([128, 512], dt.float16)
        nc.gpsimd.memset(warm, 0.0)
        st_w1 = consts.tile([128, 1], dt.float32)
        st_w1b = consts.tile([128, 1], dt.float32)
        st_l2 = consts.tile([128, 1], dt.float32)
        st_m = consts.tile([128, 1], dt.float32)
        st2 = consts.tile([128, 16], dt.float32)  # auxiliary accums
        nc.vector.memset(st2, 0.0)

        # layout-B persistent tensors
        tbx = keep.tile([128, G4, W + 2], dt.float16)  # target + dup col
        maskb = keep.tile([128, G4, W], dt.float16)

        # target load first on the sync queue (chunk 0 follows immediately;
        # the cast and everything downstream need it early)
        t32b = mp.tile([128, G4, W], dt.int32)
        nc.sync.dma_start(
            out=t32b, in_=target.ap().rearrange("(p r) w -> p r w", p=128))
        # rows 4p+4 (vertical-edge seam) as a direct DRAM load — an SBUF
        # partition-shift DMA here lands ~30us late and head-of-line
        # blocks the DVE queue
        trow4 = mp.tile([128, 1, W], dt.int32)
        nc.sync.dma_start(
            out=trow4[0:127, :, :],
            in_=target.ap().rearrange("(p r) w -> p r w", p=128)[1:128, 0:1, :])

        # mask-pipeline tiles (padded horizontal buffers)
        ehp = mp.tile([128, G4, PW], dt.float16)
        evp = mp.tile([128, G4, PW], dt.float16)
        m2p = ms.tile([128, G4, PW], dt.float16)
        m4p = ms.tile([128, G4, PW], dt.float16)
        ap_t = mp.tile([128, G4, PW], dt.float16)
        bp_t = mp.tile([128, G4, PW], dt.float16)
        # contiguous staging packs so each partition-shift is ONE DMA with
        # multi-KB per-partition descriptors (tiny per-row shifts starve
        # the pred stream with a packet storm)
        pkD = mp.tile([128, G4, W], dt.float16)  # P23a, P23b, A3, B3
        pkU = mp.tile([128, 3, W], dt.float16)   # A0, P01a, B0
        p2 = mp.tile([128, 3, W], dt.float16)    # P01b, PPa, PPb
        shD = mp.tile([128, G4, W], dt.float16)
        shU = mp.tile([128, 3, W], dt.float16)
        tshf = mp.tile([128, 1, W], dt.float16)
        dramp = ctx.enter_context(tc.tile_pool(name="dram", bufs=1,
                                               space="DRAM"))
        bnc = dramp.tile([128, 7, W], dt.float16)  # bounce for part-shifts
        a5 = mp.tile([128, G4, W], dt.float16)
        b4 = mp.tile([128, G4, W], dt.float16)

        # border memsets (neutral 0 for max-pool of {0,1} indicators)
        for t in (ehp, evp):
            nc.gpsimd.memset(t[:, :, 0:2], 0.0)
            nc.gpsimd.memset(t[:, :, 2 + W:PW], 0.0)
        nc.gpsimd.memset(evp[:, 3:4, :], 0.0)  # ev row 511 := 0 (p=127)
        nc.gpsimd.memset(m2p[:, G4 - 1, PW - 2:PW], 0.0)
        nc.gpsimd.memset(m4p[:, G4 - 1, PW - 4:PW], 0.0)
        nc.gpsimd.memset(ap_t[:, 0, 0:2], 0.0)
        nc.gpsimd.memset(bp_t[:, 0, 0:2], 0.0)
        nc.gpsimd.memset(bp_t[:, G4 - 1, PW - 2:PW], 0.0)
        nc.gpsimd.memset(shD, 0.0)
        nc.gpsimd.memset(shU, 0.0)

        # target cast int32 -> fp16 and duplicated column for eh border
        nc.vector.tensor_copy(out=tbx[:, :, 0:W], in_=t32b)
        nc.vector.tensor_copy(out=tbx[:, :, W:W + 1], in_=tbx[:, :, W - 1:W])
        nc.vector.tensor_copy(out=tshf[0:127, :, :], in_=trow4[0:127, :, :])

        # PE warmup into the future S bank (discarded by c==0's start=True)
        s_ps = sgp.tile([128, G4, W], dt.float32, tag="s")
        g_ps = sgp.tile([128, G4, W], dt.float32, tag="g")
        for _ in range(10):
            nc.tensor.matmul(s_ps[:, 0, :], ident, warm, start=True,
                             stop=True)

        # flat views over the padded buffers (borders absorb row crossings)
        FL = G4 * PW

        def flat(t):
            return t.rearrange("p r w -> p (r w)")

        ehf, evf = flat(ehp), flat(evp)
        m2f, m4f = flat(m2p), flat(m4p)
        af, bf = flat(ap_t), flat(bp_t)
        dc = lambda t, r: t[:, r, 2:2 + W]  # data columns of a padded row

        # ---------------- mask pipeline stages (layout B) ----------------
        def st_edges():
            # eh[r,j] = t[r,j] != t[r,j+1]  (col 512 duplicates 511 -> 0)
            nc.vector.tensor_tensor(
                out=ehp[:, :, 2:2 + W], in0=tbx[:, :, 0:W],
                in1=tbx[:, :, 1:W + 1], op=Alu.not_equal)
            # ev rows 4p+0..2: within partition
            nc.vector.tensor_tensor(
                out=evp[:, 0:3, 2:2 + W], in0=tbx[:, 0:3, 0:W],
                in1=tbx[:, 1:4, 0:W], op=Alu.not_equal)
            # ev row 4p+3 vs row 4(p+1) via the direct-loaded seam row
            nc.vector.tensor_tensor(
                out=evp[0:127, 3, 2:2 + W], in0=tbx[0:127, 3, 0:W],
                in1=tshf[0:127, 0, :], op=Alu.not_equal)

        def st_m2_ha():
            nc.vector.tensor_tensor(
                out=m2f[:, 0:FL - 2], in0=ehf[:, 0:FL - 2],
                in1=ehf[:, 1:FL - 1], op=Alu.max)
            # A[x] = max(m2[x-2], m2[x]) -> horizontal window [-2,+1] of eh
            nc.vector.tensor_tensor(
                out=af[:, 2:FL], in0=m2f[:, 0:FL - 2], in1=m2f[:, 2:FL],
                op=Alu.max)

        def st_m2e_m4():
            nc.vector.tensor_tensor(
                out=m2f[:, 0:FL - 2], in0=evf[:, 0:FL - 2],
                in1=evf[:, 1:FL - 1], op=Alu.max)
            nc.vector.tensor_tensor(
                out=m4f[:, 0:FL - 4], in0=m2f[:, 0:FL - 4],
                in1=m2f[:, 2:FL - 2], op=Alu.max)

        def st_hb():
            # B[x] = max(m4[x-2], ev[x+2]) -> horizontal window [-2,+2]
            nc.vector.tensor_tensor(
                out=bf[:, 2:FL - 2], in0=m4f[:, 0:FL - 4],
                in1=evf[:, 4:FL], op=Alu.max)

        def st_pk():
            nc.vector.tensor_tensor(out=pkD[:, 0, :], in0=dc(ap_t, 2),
                                    in1=dc(ap_t, 3), op=Alu.max)  # P23a
            nc.vector.tensor_tensor(out=pkD[:, 1, :], in0=dc(bp_t, 2),
                                    in1=dc(bp_t, 3), op=Alu.max)  # P23b
            nc.vector.tensor_copy(out=pkD[:, 2, :], in_=dc(ap_t, 3))  # A3
            nc.vector.tensor_copy(out=pkD[:, 3, :], in_=dc(bp_t, 3))  # B3
            nc.vector.tensor_copy(out=pkU[:, 0, :], in_=dc(ap_t, 0))  # A0
            nc.vector.tensor_tensor(out=pkU[:, 1, :], in0=dc(ap_t, 0),
                                    in1=dc(ap_t, 1), op=Alu.max)  # P01a
            nc.vector.tensor_copy(out=pkU[:, 2, :], in_=dc(bp_t, 0))  # B0
            nc.vector.tensor_tensor(out=p2[:, 0, :], in0=dc(bp_t, 0),
                                    in1=dc(bp_t, 1), op=Alu.max)  # P01b
            nc.vector.tensor_tensor(out=p2[:, 1, :], in0=pkD[:, 0, :],
                                    in1=pkU[:, 1, :], op=Alu.max)  # PPa
            nc.vector.tensor_tensor(out=p2[:, 2, :], in0=pkD[:, 1, :],
                                    in1=p2[:, 0, :], op=Alu.max)  # PPb

        def st_bounce_w():
            # partition shifts via DRAM bounce on the scalar HWDGE queue
            # (SWDGE legs measured 10-20x slower).  Emitted between exps
            # only after pk is guaranteed complete, so the ACT FIFO never
            # blocks on the pk semaphore.
            nc.scalar.dma_start(out=bnc[:, 0:4, :], in_=pkD)
            nc.scalar.dma_start(out=bnc[:, 4:7, :], in_=pkU)

        def st_bounce_r():
            # shD[p] <- pk rows of p-1; shU[p] <- pk rows of p+1
            nc.scalar.dma_start(out=shD[1:128, :, :], in_=bnc[0:127, 0:4, :])
            nc.scalar.dma_start(out=shU[0:127, :, :], in_=bnc[1:128, 4:7, :])

        def st_a5():
            # vertical 5-window of A, rows 4p+r' for r'=0..3
            nc.vector.tensor_tensor(out=a5[:, 0, :], in0=pkU[:, 1, :],
                                    in1=dc(ap_t, 2), op=Alu.max)
            nc.vector.tensor_tensor(out=a5[:, 0, :], in0=a5[:, 0, :],
                                    in1=shD[:, 0, :], op=Alu.max)
            nc.vector.tensor_tensor(out=a5[:, 1, :], in0=p2[:, 1, :],
                                    in1=shD[:, 2, :], op=Alu.max)
            nc.vector.tensor_tensor(out=a5[:, 2, :], in0=p2[:, 1, :],
                                    in1=shU[:, 0, :], op=Alu.max)
            nc.vector.tensor_tensor(out=a5[:, 3, :], in0=dc(ap_t, 1),
                                    in1=pkD[:, 0, :], op=Alu.max)
            nc.vector.tensor_tensor(out=a5[:, 3, :], in0=a5[:, 3, :],
                                    in1=shU[:, 1, :], op=Alu.max)

        def st_b4():
            # vertical 4-window [-2,+1] of B
            nc.vector.tensor_tensor(out=b4[:, 0, :], in0=p2[:, 0, :],
                                    in1=shD[:, 1, :], op=Alu.max)
            nc.vector.tensor_tensor(out=b4[:, 1, :], in0=p2[:, 0, :],
                                    in1=dc(bp_t, 2), op=Alu.max)
            nc.vector.tensor_tensor(out=b4[:, 1, :], in0=b4[:, 1, :],
                                    in1=shD[:, 3, :], op=Alu.max)
            nc.vector.tensor_copy(out=b4[:, 2, :], in_=p2[:, 2, :])
            nc.vector.tensor_tensor(out=b4[:, 3, :], in0=dc(bp_t, 1),
                                    in1=pkD[:, 1, :], op=Alu.max)
            nc.vector.tensor_tensor(out=b4[:, 3, :], in0=b4[:, 3, :],
                                    in1=shU[:, 2, :], op=Alu.max)

        def st_merge():
            # mask = max(A5, B4); msum accumulated in the same op
            nc.vector.scalar_tensor_tensor(
                out=maskb, in0=a5, scalar=0.0, in1=b4,
                op0=Alu.add, op1=Alu.max, accum_out=st_m)

        # DVE stage work front-loaded (chunks 0-3; no stage ever waits on
        # a slow DMA mid-stream).  The vertical stages A5/B4/merge are
        # emitted AFTER the class loop: the bounced shifts land ~45-50us,
        # and placing their consumers post-loop keeps the in-order DVE
        # queue from head-of-line blocking the per-class work.
        stage_at = {
            0: [st_edges],
            1: [st_m2_ha, st_m2e_m4],
            2: [st_hb, st_pk],
            9: [st_a5],
            10: [st_b4],
        }

        # ---------------- class loop (layout B), stages interleaved -------
        tb = tbx[:, :, 0:W]
        KD = G4 - POOL_EQ_ROWS  # rows of eq on DVE
        starts = list(range(0, C, CHUNK))
        for k, c0 in enumerate(starts):
            nct = min(CHUNK, C - c0)
            p_t = ppool.tile([128, nct, G4, W], dt.float32, tag="p")
            nc.sync.dma_start(
                out=p_t,
                in_=pred.ap()[c0:c0 + nct].rearrange(
                    "c (p r) w -> p c r w", p=128))

            e_t = epool.tile([128, nct, G4, W], dt.float16, tag="e")
            nc.scalar.activation(out=e_t, in_=p_t, func=Act.Exp)
            for i in range(nct):
                c = c0 + i
                eqd = qpool.tile([128, KD, W], dt.float16, tag="qd")
                nc.vector.tensor_scalar(
                    out=eqd, in0=tb[:, 0:KD, :], scalar1=float(c),
                    scalar2=None, op0=Alu.is_equal)
                if POOL_EQ_ROWS:
                    eqp = qpool.tile([128, POOL_EQ_ROWS, W], dt.float16,
                                     tag="qp")
                    nc.gpsimd.tensor_scalar(
                        out=eqp, in0=tb[:, KD:G4, :], scalar1=float(c),
                        scalar2=None, op0=Alu.is_equal)
                o_t = opool.tile([128, G4, W], dt.float16, tag="o")
                nc.vector.tensor_tensor(
                    out=o_t[:, 0:KD, :], in0=eqd, in1=e_t[:, i, 0:KD, :],
                    op=Alu.mult)
                if POOL_EQ_ROWS:
                    nc.vector.tensor_tensor(
                        out=o_t[:, KD:G4, :], in0=eqp,
                        in1=e_t[:, i, KD:G4, :], op=Alu.mult)
                for j in range(G4):
                    nc.tensor.matmul(
                        s_ps[:, j, :], ident, e_t[:, i, j, :],
                        start=(c == 0), stop=(c == C - 1))
                for j in range(G4):
                    nc.tensor.matmul(
                        g_ps[:, j, :], ident, o_t[:, j, :],
                        start=(c == 0), stop=(c == C - 1))
            for st in stage_at.get(k, ()):
                st()
            if k == 6:
                st_bounce_w()   # pk done by ~chunk 4; no ACT FIFO stall
            elif k == 8:
                st_bounce_r()   # write completed by ~chunk 7

        st_merge()

        # ---------------- finals (half-split, fp16 Ln) ----------------
        l1 = fin.tile([128, G4, W], dt.float16)
        lg = fin.tile([128, G4, W], dt.float16)
        j1 = jpool.tile([128, G4, W], dt.float16, tag="junk")
        j2 = jpool.tile([128, G4, W], dt.float16, tag="junk")
        hh = G4 // 2
        nc.scalar.activation(out=l1[:, 0:hh, :], in_=s_ps[:, 0:hh, :],
                             func=Act.Ln)
        nc.vector.scalar_tensor_tensor(
            out=j1[:, 0:hh, :], in0=l1[:, 0:hh, :], scalar=0.0,
            in1=maskb[:, 0:hh, :], op0=Alu.add, op1=Alu.mult,
            accum_out=st_w1)
        nc.scalar.activation(out=l1[:, hh:G4, :], in_=s_ps[:, hh:G4, :],
                             func=Act.Ln)
        nc.vector.scalar_tensor_tensor(
            out=j1[:, hh:G4, :], in0=l1[:, hh:G4, :], scalar=0.0,
            in1=maskb[:, hh:G4, :], op0=Alu.add, op1=Alu.mult,
            accum_out=st_w1b)
        nc.scalar.activation(out=lg[:, 0:hh, :], in_=g_ps[:, 0:hh, :],
                             func=Act.Ln)
        nc.vector.scalar_tensor_tensor(
            out=j2[:, 0:hh, :], in0=lg[:, 0:hh, :], scalar=0.0,
            in1=maskb[:, 0:hh, :], op0=Alu.add, op1=Alu.mult,
            accum_out=st_l2)
        nc.scalar.activation(out=lg[:, hh:G4, :], in_=g_ps[:, hh:G4, :],
                             func=Act.Ln)
        nc.vector.scalar_tensor_tensor(
            out=j2[:, hh:G4, :], in0=lg[:, hh:G4, :], scalar=0.0,
            in1=maskb[:, hh:G4, :], op0=Alu.add, op1=Alu.mult,
            accum_out=st2[:, 0:1])

        # partition reductions — reuse the S bank (fully consumed by l1)
        red = s_ps[0:1, 0, 0:32]
        nc.tensor.matmul(red[:, 0:1], ones, st_w1, start=True, stop=True)
        nc.tensor.matmul(red[:, 1:2], ones, st_l2, start=True, stop=True)
        nc.tensor.matmul(red[:, 2:3], ones, st_m, start=True, stop=True)
        nc.tensor.matmul(red[:, 3:4], ones, st_w1b, start=True, stop=True)
        nc.tensor.matmul(red[:, 8:24], ones, st2, start=True, stop=True)
        outsb = consts.tile([1, 32], dt.float32)
        nc.vector.memset(outsb, 0.0)
        nc.vector.tensor_copy(out=outsb[:, 0:4], in_=red[:, 0:4])
        nc.vector.tensor_copy(out=outsb[:, 8:24], in_=red[:, 8:24])
        nc.sync.dma_start(out=out.ap(), in_=outsb)

    nc.compile()
    return nc


def get_nc():
    if "nc" not in _CACHE:
        _CACHE["nc"] = _build_nc()
    return _CACHE["nc"]


def _combine(outs):
    """outs: list of per-core [1,32] float32 -> scalar loss."""
    per_sample = []
    for o in outs:
        w1, l2, msum = float(o[0, 0]), float(o[0, 1]), float(o[0, 2])
        w1 += float(o[0, 3])  # second half of the W1 accumulation
        l2 += float(o[0, 8:24].sum())  # second half of the L2 accumulation
        wsum = w1 - l2
        if msum > 0:
            per_sample.append(wsum / max(msum, 1.0))
        else:
            per_sample.append(wsum / float(H * W))
    return np.float32(np.mean(per_sample))


def kernel(pred, target):
    from concourse.bass_utils import run_bass_kernel_spmd

    pred = np.ascontiguousarray(pred, dtype=np.float32)
    target = np.ascontiguousarray(target, dtype=np.int32)
    assert pred.shape == (B, C, H, W) and target.shape == (B, H, W)

    nc = get_nc()
    in_maps = [{"pred": pred[b], "target": target[b]} for b in range(B)]
    res = run_bass_kernel_spmd(nc, in_maps, core_ids=list(range(N_CORES)))
    outs = [res.results[b]["out"] for b in range(B)]
    return np.asarray(_combine(outs), dtype=np.float32)


# revision 44
# speedup vs baseline: 1.6385x; 1.6385x over previous
"""Trainium2 Bass kernel for BoundaryLoss (data-parallel over batch).

Math (per batch sample b):
  mask  = boundary mask of target = (maxpool5x5(t) != minpool5x5(t)) with
          cv2-style clipped windows (OOB ignored).  Equals the reference's
          per-class dilate/erode union because a 5x5 window is non-uniform
          iff some class boundary passes through it.
  ce    = logsumexp_c(pred) - pred[t]
  wsum  = sum(mask * ce);  msum = sum(mask)
  per_sample = msum > 0 ? wsum/max(msum,1) : wsum/(H*W);  out = mean_b

Device algorithm (one sample per core):
  - pred streams in "layout B" [128, (4 rows, 512)] (partition p = rows
    4p..4p+3) giving 8KB-contiguous DMA runs (~400+ GB/s measured) — the
    21 MB pred stream is the roofline for this kernel.
  - S = sum_c exp(pred_c): exp on ACT (fp16 out), summed over classes by
    identity-matmul PSUM accumulation on TensorE.
  - picked = pred[t], mask-weighted, is gathered two ways:
      early classes (before the mask is ready): eq=(t==c) on DVE 4x, then
        copy_predicated G[t==c] = e_c into SBUF; finals add
        sum(mask*ln(G)) (G init to 1 so untouched pixels contribute 0).
      late classes (K0 < C only; currently disabled, K0=C — holding raw
        pred tiles for the mask-gated fused-stt path stalled the DMA
        stream more than the DVE savings were worth).
  - boundary mask concurrently in "layout A" [128, (4, 512)] (partition =
    row g*128+p): horizontal 5-max/min via 3 shifted tensor_tensor ops,
    PE-transpose 128x128 blocks (PSUM), vertical pools in transposed
    space, compare, PE-transpose back, bounce through DRAM into layout B.
    Emission is interleaved between class chunks so every engine's
    (statically ordered) instruction stream stays dependency-ready.
  - finals: sum(mask*ln(S)) and sum(mask*ln(G)) via stt accum; msum via
    ACT accum; partition-reduce via ones-matmuls; DMA out [1,32].
Host combines the per-core outputs.
"""

import numpy as np

B = 8
C = 21
H = 512
W = 512
N_CORES = 8
CHUNK = 2  # pred planes per DMA
K0 = 21  # classes [0, K0) use copy_predicated; [K0, C) use masked stt accum
PW = 520  # padded width of pooling buffers; data cols [2, 514)
G4 = 4  # row groups (H = G4 * 128)

_CACHE = {}


def _build_nc():
    from contextlib import ExitStack

    import concourse.bacc as bacc
    import concourse.tile as tile
    from concourse import mybir
    from concourse.masks import make_identity

    dt = mybir.dt
    Alu = mybir.AluOpType
    Act = mybir.ActivationFunctionType

    nc = bacc.Bacc("TRN2", target_bir_lowering=False, debug=False,
                   num_devices=N_CORES)

    pred = nc.dram_tensor("pred", [C, H, W], dt.float32, kind="ExternalInput")
    target = nc.dram_tensor("target", [H, W], dt.int32, kind="ExternalInput")
    out = nc.dram_tensor("out", [1, 32], dt.float32, kind="ExternalOutput")

    with tile.TileContext(nc) as tc, ExitStack() as ctx:
        consts = ctx.enter_context(tc.tile_pool(name="consts", bufs=1))
        keep = ctx.enter_context(tc.tile_pool(name="keep", bufs=1))
        mp = ctx.enter_context(tc.tile_pool(name="maskpool", bufs=1))
        ms = ctx.enter_context(tc.tile_pool(name="maskscratch", bufs=1))
        ppool = ctx.enter_context(tc.tile_pool(name="pp", bufs=3))
        epool = ctx.enter_context(tc.tile_pool(name="ep", bufs=3))
        qpool = ctx.enter_context(tc.tile_pool(name="qp", bufs=4))
        jpool = ctx.enter_context(tc.tile_pool(name="jp", bufs=2))
        opool = ctx.enter_context(tc.tile_pool(name="op", bufs=4))
        fin = ctx.enter_context(tc.tile_pool(name="fin", bufs=1))
        dramp = ctx.enter_context(tc.tile_pool(name="dram", bufs=1,
                                               space="DRAM"))
        mps = ctx.enter_context(tc.tile_pool(name="mpsum", bufs=1,
                                             space="PSUM"))
        sgp = ctx.enter_context(tc.tile_pool(name="sgpsum", bufs=1,
                                             space="PSUM"))

        ident = consts.tile([128, 128], dt.float16)
        make_identity(nc, ident)
        ones = consts.tile([128, 1], dt.float32)
        nc.gpsimd.memset(ones, 1.0)
        warm = consts.tile([128, 512], dt.float16)
        nc.gpsimd.memset(warm, 0.0)
        st_w1 = consts.tile([128, 1], dt.float32)
        st_l2 = consts.tile([128, 1], dt.float32)
        st_m = consts.tile([128, 1], dt.float32)
        st2 = consts.tile([128, 16], dt.float32)  # auxiliary l2 accums
        nc.vector.memset(st2, 0.0)

        # layout-B tensors
        tb = keep.tile([128, G4, W], dt.float16)      # target as fp16
        maskb = keep.tile([128, G4, W], dt.float16)   # mask (from bounce)
        g_sb = keep.tile([128, 1, W], dt.float16)     # r=3 gather | 1.0
        mask_dram = dramp.tile([H, W], dt.float16)

        # ---------------- early loads ----------------
        t32 = mp.tile([128, G4, W], dt.int32)
        nc.sync.dma_start(
            out=t32, in_=target.ap().rearrange("(g p) w -> p g w", p=128))
        t32b = mp.tile([128, G4, W], dt.int32, tag="t32b")
        nc.sync.dma_start(
            out=t32b, in_=target.ap().rearrange("(p r) w -> p r w", p=128))
        nc.vector.tensor_copy(out=tb, in_=t32b)
        nc.gpsimd.memset(g_sb, 1.0)

        # PE warmup into the future S bank (discarded by c==0's start=True)
        s_ps = sgp.tile([128, G4, W], dt.float32, tag="s")
        g_ps = sgp.tile([128, 3, W], dt.float32, tag="g")
        for _ in range(10):
            nc.tensor.matmul(s_ps[:, 0, :], ident, warm, start=True,
                             stop=True)

        # ---------------- mask pipeline stages (layout A) ----------------
        xmax = mp.tile([128, G4, PW], dt.float16, tag="xmax")
        xmin = mp.tile([128, G4, PW], dt.float16, tag="xmin")
        xt = mp.tile([128, G4, PW], dt.float16, tag="xt")
        xnt = mp.tile([128, G4, PW], dt.float16, tag="xnt")
        for t in (xmax, xt):
            nc.gpsimd.memset(t[:, :, 0:2], -1.0)
            nc.gpsimd.memset(t[:, :, 2 + W:PW], -1.0)
        for t in (xmin, xnt):
            nc.gpsimd.memset(t[:, :, 0:2], 99.0)
            nc.gpsimd.memset(t[:, :, 2 + W:PW], 99.0)
        hx = mp.tile([128, G4, W], dt.float16, tag="hx")
        hn = mp.tile([128, G4, W], dt.float16, tag="hn")
        vx = mp.tile([128, G4, W], dt.float16, tag="hx")   # reuse slot
        vn = mp.tile([128, G4, W], dt.float16, tag="hn")   # reuse slot
        maskt = mp.tile([128, G4, W], dt.float16, tag="maskt")
        mask_a = mp.tile([128, G4, W], dt.float16, tag="maska")

        def pool5(src, op, dst):
            m2 = ms.tile([128, G4, PW], dt.float16, tag="m2")
            m4 = ms.tile([128, G4, PW], dt.float16, tag="m4")
            nc.vector.tensor_tensor(
                out=m2[:, :, 0:PW - 1],
                in0=src[:, :, 0:PW - 1], in1=src[:, :, 1:PW], op=op)
            nc.vector.tensor_tensor(
                out=m4[:, :, 0:PW - 3],
                in0=m2[:, :, 0:PW - 3], in1=m2[:, :, 2:PW - 1], op=op)
            nc.vector.tensor_tensor(
                out=dst, in0=m4[:, :, 0:W], in1=src[:, :, 4:4 + W], op=op)

        def tpose_in(src, dst):
            for q in range(4):
                tq = mps.tile([128, 512], dt.float16, tag="tq")
                for g in range(4):
                    nc.tensor.transpose(
                        tq[:, g * 128:(g + 1) * 128],
                        src[:, g, q * 128:(q + 1) * 128], ident)
                nc.scalar.copy(out=dst[:, q, 2:2 + W], in_=tq)

        def st_casts():
            nc.vector.tensor_copy(out=xmax[:, :, 2:2 + W], in_=t32)
            nc.vector.tensor_copy(out=xmin[:, :, 2:2 + W], in_=t32)

        def st_neq():
            nc.vector.tensor_tensor(out=maskt, in0=vx, in1=vn,
                                    op=Alu.not_equal)
            junk_m = ms.tile([128, G4, W], dt.float16, tag="junkm")
            nc.scalar.activation(out=junk_m, in_=maskt, func=Act.Copy,
                                 accum_out=st_m)

        def st_back():
            for g in range(4):
                tg = mps.tile([128, 512], dt.float16, tag="tq")
                for q in range(4):
                    nc.tensor.transpose(
                        tg[:, q * 128:(q + 1) * 128],
                        maskt[:, q, g * 128:(g + 1) * 128], ident)
                nc.scalar.copy(out=mask_a[:, g, :], in_=tg)

        def st_bounce():
            nc.gpsimd.dma_start(
                out=mask_dram[:].rearrange("(g p) w -> p g w", p=128),
                in_=mask_a)
            nc.gpsimd.dma_start(
                out=maskb,
                in_=mask_dram[:].rearrange("(p r) w -> p r w", p=128))

        def st_tt2():
            # tt2b = (t+1) * mask, in layout B
            nc.vector.scalar_tensor_tensor(
                out=tt2b, in0=tb, scalar=1.0, in1=maskb,
                op0=Alu.add, op1=Alu.mult)

        stages = [
            st_casts,
            lambda: pool5(xmax, Alu.max, hx),
            lambda: pool5(xmin, Alu.min, hn),
            lambda: tpose_in(hx, xt),
            lambda: tpose_in(hn, xnt),
            lambda: pool5(xt, Alu.max, vx),
            lambda: pool5(xnt, Alu.min, vn),
            st_neq,
            st_back,
            st_bounce,
        ]

        # ---------------- class loop (layout B), stages interleaved -------
        starts = list(range(0, C, CHUNK))
        for k, c0 in enumerate(starts):
            if k < len(stages):
                stages[k]()
            nct = min(CHUNK, C - c0)
            p_t = ppool.tile([128, nct, G4, W], dt.float32, tag="p")
            nc.sync.dma_start(
                out=p_t,
                in_=pred.ap()[c0:c0 + nct].rearrange(
                    "c (p r) w -> p c r w", p=128))
            e_t = epool.tile([128, nct, G4, W], dt.float16, tag="e")
            nc.scalar.activation(out=e_t, in_=p_t, func=Act.Exp)
            for i in range(nct):
                c = c0 + i
                eq_t = qpool.tile([128, G4, W], dt.uint16, tag="q")
                nc.vector.tensor_scalar(
                    out=eq_t, in0=tb, scalar1=float(c), scalar2=None,
                    op0=Alu.is_equal)
                # rows 0..2: gather via 2x multiply + identity matmul
                o_t = opool.tile([128, 3, W], dt.float16, tag="o")
                nc.vector.tensor_tensor(
                    out=o_t, in0=eq_t[:, 0:3, :], in1=e_t[:, i, 0:3, :],
                    op=Alu.mult)
                # row 3: gather via predicated overwrite (1x but quarter-FD)
                nc.vector.copy_predicated(out=g_sb[:, 0, :],
                                          mask=eq_t[:, 3, :],
                                          data=e_t[:, i, 3, :])
                for j in range(4):
                    nc.tensor.matmul(
                        s_ps[:, j, :], ident, e_t[:, i, j, :],
                        start=(c == 0), stop=(c == C - 1))
                for j in range(3):
                    nc.tensor.matmul(
                        g_ps[:, j, :], ident, o_t[:, j, :],
                        start=(c == 0), stop=(c == C - 1))
        for k in range(len(starts), len(stages)):
            stages[k]()

        # ---------------- finals ----------------
        l1 = fin.tile([128, G4, W], dt.float32)
        nc.scalar.activation(out=l1, in_=s_ps, func=Act.Ln)
        lg3 = fin.tile([128, 3, W], dt.float32)
        nc.scalar.activation(out=lg3, in_=g_ps, func=Act.Ln)
        lg4 = fin.tile([128, 1, W], dt.float32)
        nc.scalar.activation(out=lg4, in_=g_sb, func=Act.Ln)

        j1 = jpool.tile([128, G4, W], dt.float32, tag="junk")
        nc.vector.scalar_tensor_tensor(
            out=j1, in0=l1, scalar=0.0, in1=maskb,
            op0=Alu.add, op1=Alu.mult, accum_out=st_w1)
        j2 = jpool.tile([128, G4, W], dt.float32, tag="junk")
        nc.vector.scalar_tensor_tensor(
            out=j2[:, 0:3, :], in0=lg3, scalar=0.0, in1=maskb[:, 0:3, :],
            op0=Alu.add, op1=Alu.mult, accum_out=st_l2)
        nc.vector.scalar_tensor_tensor(
            out=j2[:, 3:4, :], in0=lg4, scalar=0.0, in1=maskb[:, 3:4, :],
            op0=Alu.add, op1=Alu.mult, accum_out=st2[:, 0:1])

        # partition reductions — reuse the S bank (fully consumed by l1)
        red = s_ps[0:1, 0, 0:32]
        nc.tensor.matmul(red[:, 0:1], ones, st_w1, start=True, stop=True)
        nc.tensor.matmul(red[:, 1:2], ones, st_l2, start=True, stop=True)
        nc.tensor.matmul(red[:, 2:3], ones, st_m, start=True, stop=True)
        nc.tensor.matmul(red[:, 8:24], ones, st2, start=True, stop=True)
        outsb = consts.tile([1, 32], dt.float32)
        nc.vector.memset(outsb, 0.0)
        nc.vector.tensor_copy(out=outsb[:, 0:3], in_=red[:, 0:3])
        nc.vector.tensor_copy(out=outsb[:, 8:24], in_=red[:, 8:24])
        nc.sync.dma_start(out=out.ap(), in_=outsb)

    nc.compile()
    return nc


def get_nc():
    if "nc" not in _CACHE:
        _CACHE["nc"] = _build_nc()
    return _CACHE["nc"]


def _combine(outs):
    """outs: list of per-core [1,32] float32 -> scalar loss."""
    per_sample = []
    for o in outs:
        w1, l2, msum = float(o[0, 0]), float(o[0, 1]), float(o[0, 2])
        l2 += float(o[0, 8:24].sum())  # auxiliary l2 partial sums
        wsum = w1 - l2
        if msum > 0:
            per_sample.append(wsum / max(msum, 1.0))
        else:
            per_sample.append(wsum / float(H * W))
    return np.float32(np.mean(per_sample))


def kernel(pred, target):
    from concourse.bass_utils import run_bass_kernel_spmd

    pred = np.ascontiguousarray(pred, dtype=np.float32)
    target = np.ascontiguousarray(target, dtype=np.int32)
    assert pred.shape == (B, C, H, W) and target.shape == (B, H, W)

    nc = get_nc()
    in_maps = [{"pred": pred[b], "target": target[b]} for b in range(B)]
    res = run_bass_kernel_spmd(nc, in_maps, core_ids=list(range(N_CORES)))
    outs = [res.results[b]["out"] for b in range(B)]
    return np.asarray(_combine(outs), dtype=np.float32)

